# revision 20
# baseline (speedup 1.0000x reference)
"""Complex per-mode matmul: out[b,o,x,y] = sum_i in[b,i,x,y] * w[i,o,x,y] (complex).

Shapes (hardcoded): input [32,128,64,65,2] f32, weight [128,128,64,65,2] f32,
output [32,128,64,65,2] f32, where the trailing 2 is (real, imag).

Strategy:
  - Shard the 64 x-modes across 8 cores (8 per core). Contraction is over
    in_channels for each (x,y) independently, so this needs zero replication
    and no collectives: per-core I/O is 1/8 of everything.
  - Per mode (x,y): psum[o, c*32+b] accumulates two matmuls
        MM1: lhsT=Wr[i,o] (128 cols), rhs cols (Xr[i,b] | Xi[i,b]) blocks -> (Wr.Xr | Wr.Xi)
        MM2: lhsT=Wi[i,o],           rhs cols (-Xi[i,b] | Xr[i,b]) blocks -> (-Wi.Xi | Wi.Xr)
    giving out_r = Wr.Xr - Wi.Xi in the low 32 cols, out_i = Wr.Xi + Wi.Xr in
    the high 32. The -Xi block is produced on-device by one DVE scalar-mul per
    x-slice (cheaper than shipping a third X copy from HBM).
  - Inputs fp16 (PSUM accumulates fp32); output stored fp16 and upcast on host.
  - Host pre-transposes operands so every DMA moves large contiguous
    per-partition lines:
      cin layout [x][i (part)][w: c(2),y(65),o(128) | xr: y(65),b(32) | xi: y(65),b(32)]
      out layout [o (part)][x][b(32), y(65), c(2)]   fp16
  - This walrus build fits only ONE sync wait per hardware instruction; a
    post-pass splits any extra waits into standalone EventSemaphore
    instructions on the same engine queue (the wait-carrier bacc uses).
"""

import numpy as np

B, CIN, COUT, M1, M2 = 32, 128, 128, 64, 65
NCORES = 8
XPC = M1 // NCORES  # x-slices per core
MPG = 8  # modes per PSUM bank (8 * 64 cols = 512 = one bank)


def _split_excess_waits(nc, mybir):
    """Walrus codegen fits one sync wait per instruction; move extras onto
    EventSemaphore instructions inserted just before, on the same engine."""
    n = 0
    for fn in nc.m.functions:
        for blk in fn.blocks:
            out = []
            for inst in blk.instructions:
                si = inst.sync_info
                if si is not None and si.on_wait and len(si.on_wait) > 1:
                    waits = list(si.on_wait)
                    for w in waits[:-1]:
                        ev = mybir.InstEventSemaphore(
                            name=f"evsplit_{n}",
                            engine=inst.engine,
                            ins=[],
                            outs=[],
                            sync_info=mybir.SyncInfo(on_wait=[w], on_update=[]),
                            bass_nofuse=True,
                        )
                        n += 1
                        nc.register_instruction(ev)
                        out.append(ev)
                    si.on_wait = [waits[-1]]
                out.append(inst)
            blk.instructions = out


def build_nc(xpc=XPC, b=B, yc=M2, cout=COUT):
    import concourse.bass as bass
    import concourse.mybir as mybir
    from concourse.tile import TileContext

    dt = mybir.dt.float16
    f32 = mybir.dt.float32
    WW = 2 * yc * cout  # weight cols per slice
    XB = b * yc  # one x block (b, y)
    nc = bass.Bass()
    cin = nc.dram_tensor("cin", [xpc, CIN, WW + 2 * XB], dt, kind="ExternalInput")
    out = nc.dram_tensor("out", [cout, max(1, xpc // 2), 2 * b * yc * 2], dt, kind="ExternalOutput")

    groups = [(g0, min(MPG, yc - g0)) for g0 in range(0, yc, MPG)]

    with TileContext(nc) as tc:
        with (
            tc.tile_pool(name="cpool", bufs=2) as cpool,
            tc.tile_pool(name="opool", bufs=2) as opool,
            tc.tile_pool(name="ppool", bufs=4, space="PSUM") as ppool,
        ):
            OW = b * yc * 2  # out cols per slice
            otile = None
            for x in range(xpc):
                # ctile: [ w (WW) | xr (XB) | xi (XB) | -xi (XB, ACT-written) ]
                ctile = cpool.tile([CIN, WW + 3 * XB], dt, name="ctile")
                nc.sync.dma_start(out=ctile[:, : WW + 2 * XB], in_=cin[x])
                # -xi block on DVE: using ScalarE here pulls in its activation
                # tables as ~8us of static DMA before the kernel starts
                nc.vector.tensor_scalar_mul(
                    ctile[:, WW + 2 * XB :], ctile[:, WW + XB : WW + 2 * XB], -1.0
                )
                wv = ctile[:, :WW].rearrange("p (c y o) -> p c y o", c=2, y=yc)
                xv = ctile[:, WW:].rearrange("p (c y b) -> p c y b", c=3, y=yc)
                # otile col = y*64 + c*32 + b: exactly the psum layout, so the
                # copyback is a flat contiguous copy; host untangles for free.
                # One otile spans two x-slices so out-DMAs are 2x larger.
                if x % 2 == 0:
                    otile = opool.tile([cout, 2 * OW], dt, name="otile")
                obase = (x % 2) * OW
                for y0, gs in groups:
                    ptile = ppool.tile([cout, 512], f32, name="ptile")
                    for m in range(gs):
                        y = y0 + m
                        ps = ptile[:, m * 2 * b : (m + 1) * 2 * b]
                        nc.tensor.matmul(
                            ps, wv[:, 0, y, :], xv[:, 0:2, y, :],
                            start=True, stop=False,
                        )
                        nc.tensor.matmul(
                            ps, wv[:, 1, y, :], xv[:, 2::-2, y, :],
                            start=False, stop=True,
                        )
                    nc.vector.tensor_copy(
                        out=otile[:, obase + y0 * 2 * b : obase + (y0 + gs) * 2 * b],
                        in_=ptile[:, : gs * 2 * b],
                    )
                # out-DMAs ride the GPSIMD SWDGE queue: SP keeps input DMAs,
                # ACT keeps the negates (else a waiting out-DMA head-of-line
                # blocks the next negate and stalls PE); the last pair is
                # split so slice 6's output ships while slice 7 computes
                if x == xpc - 2 and xpc >= 2:
                    nc.gpsimd.dma_start(out=out[:, x // 2, :OW], in_=otile[:, :OW])
                elif x == xpc - 1 and xpc >= 2:
                    nc.gpsimd.dma_start(out=out[:, x // 2, OW:], in_=otile[:, OW:])
                elif x % 2 == 1:
                    nc.gpsimd.dma_start(out=out[:, x // 2, :], in_=otile)

    _split_excess_waits(nc, mybir)
    return nc


def prep_inputs(input, weight):
    """Host-side re-layout + fp16 cast. Returns combined [64, 128, 16640+4160]."""
    # weight [i,o,x,y,c] -> [x,i,c,y,o]
    w16 = weight.transpose(2, 0, 4, 3, 1).astype(np.float16)
    w16 = w16.reshape(M1, CIN, 2 * M2 * COUT)
    xr = input[..., 0]
    xi = input[..., 1]
    st = np.stack([xr, xi], axis=0)  # [c,b,i,x,y]
    x16 = st.transpose(3, 2, 0, 4, 1).astype(np.float16)  # [x,i,c,y,b]
    x16 = x16.reshape(M1, CIN, 2 * B * M2)
    return np.concatenate([w16, x16], axis=2)


def gather_output(per_core):
    """per_core: list of 8 arrays [cout, xpc//2, 2*yc*2*b] fp16 -> [B, COUT, M1, M2, 2] f32."""
    out = np.empty((B, COUT, M1, M2, 2), np.float32)
    for k, arr in enumerate(per_core):
        a = arr.reshape(COUT, XPC, M2, 2, B)  # [o, x, y, c, b]
        out[:, :, k * XPC : (k + 1) * XPC] = a.transpose(4, 0, 1, 2, 3)
    return out


_NC = None
TRACE = False  # test harness can set True to collect a HW profile
LAST_RESULTS = None


def kernel(input, weight):
    global _NC, LAST_RESULTS
    from concourse.bass_utils import run_bass_kernel_spmd

    if _NC is None:
        _NC = build_nc()
    c16 = prep_inputs(np.asarray(input), np.asarray(weight))
    in_maps = [
        {"cin": np.ascontiguousarray(c16[k * XPC : (k + 1) * XPC])}
        for k in range(NCORES)
    ]
    res = run_bass_kernel_spmd(_NC, in_maps, core_ids=list(range(NCORES)), trace=TRACE)
    LAST_RESULTS = res
    return gather_output([r["out"] for r in res.results])


# revision 21
# speedup vs baseline: 1.0535x; 1.0535x over previous
"""Complex per-mode matmul: out[b,o,x,y] = sum_i in[b,i,x,y] * w[i,o,x,y] (complex).

Shapes (hardcoded): input [32,128,64,65,2] f32, weight [128,128,64,65,2] f32,
output [32,128,64,65,2] f32, where the trailing 2 is (real, imag).

Strategy:
  - Shard the 64 x-modes across 8 cores (8 per core). Contraction is over
    in_channels for each (x,y) independently, so this needs zero replication
    and no collectives: per-core I/O is 1/8 of everything.
  - Per mode (x,y): psum[o, c*32+b] accumulates two matmuls
        MM1: lhsT=Wr[i,o] (128 cols), rhs cols (Xr[i,b] | Xi[i,b]) blocks -> (Wr.Xr | Wr.Xi)
        MM2: lhsT=Wi[i,o],           rhs cols (-Xi[i,b] | Xr[i,b]) blocks -> (-Wi.Xi | Wi.Xr)
    giving out_r = Wr.Xr - Wi.Xi in the low 32 cols, out_i = Wr.Xi + Wi.Xr in
    the high 32. The -Xi block is produced on-device by one DVE scalar-mul per
    x-slice (cheaper than shipping a third X copy from HBM).
  - Inputs fp16 (PSUM accumulates fp32); output stored fp16 and upcast on host.
  - Host pre-transposes operands so every DMA moves large contiguous
    per-partition lines:
      cin layout [x][i (part)][w: c(2),y(65),o(128) | xr: y(65),b(32) | xi: y(65),b(32)]
      out layout [o (part)][x][b(32), y(65), c(2)]   fp16
  - This walrus build fits only ONE sync wait per hardware instruction; a
    post-pass splits any extra waits into standalone EventSemaphore
    instructions on the same engine queue (the wait-carrier bacc uses).
"""

import numpy as np

B, CIN, COUT, M1, M2 = 32, 128, 128, 64, 65
NCORES = 8
XPC = M1 // NCORES  # x-slices per core
MPG = 8  # modes per PSUM bank (8 * 64 cols = 512 = one bank)


def _split_excess_waits(nc, mybir):
    """Walrus codegen fits one sync wait per instruction; move extras onto
    EventSemaphore instructions inserted just before, on the same engine."""
    n = 0
    for fn in nc.m.functions:
        for blk in fn.blocks:
            out = []
            for inst in blk.instructions:
                si = inst.sync_info
                if si is not None and si.on_wait and len(si.on_wait) > 1:
                    waits = list(si.on_wait)
                    for w in waits[:-1]:
                        ev = mybir.InstEventSemaphore(
                            name=f"evsplit_{n}",
                            engine=inst.engine,
                            ins=[],
                            outs=[],
                            sync_info=mybir.SyncInfo(on_wait=[w], on_update=[]),
                            bass_nofuse=True,
                        )
                        n += 1
                        nc.register_instruction(ev)
                        out.append(ev)
                    si.on_wait = [waits[-1]]
                out.append(inst)
            blk.instructions = out


def build_nc(xpc=XPC, b=B, yc=M2, cout=COUT):
    import concourse.bass as bass
    import concourse.mybir as mybir
    from concourse.tile import TileContext
    from concourse.tile_rust import add_dep_helper

    dt = mybir.dt.float16
    f32 = mybir.dt.float32
    WW = 2 * yc * cout  # weight cols per slice
    XB = b * yc  # one x block (b, y)
    nc = bass.Bass()
    cin = nc.dram_tensor("cin", [xpc, CIN, WW + 2 * XB], dt, kind="ExternalInput")
    out = nc.dram_tensor("out", [cout, max(1, xpc // 2), 2 * b * yc * 2], dt, kind="ExternalOutput")

    groups = [(g0, min(MPG, yc - g0)) for g0 in range(0, yc, MPG)]

    with TileContext(nc) as tc:
        with (
            tc.tile_pool(name="cpool", bufs=2) as cpool,
            tc.tile_pool(name="opool", bufs=3) as opool,
            tc.tile_pool(name="ppool", bufs=4, space="PSUM") as ppool,
        ):
            OW = b * yc * 2  # out cols per slice
            otile = None
            in_dmas = []
            deferred = []
            for x in range(xpc):
                # ctile: [ w (WW) | xr (XB) | xi (XB) | -xi (XB, ACT-written) ]
                ctile = cpool.tile([CIN, WW + 3 * XB], dt, name="ctile")
                in_dmas.append(nc.sync.dma_start(out=ctile[:, : WW + 2 * XB], in_=cin[x]))
                # -xi block on DVE: using ScalarE here pulls in its activation
                # tables as ~8us of static DMA before the kernel starts
                nc.vector.tensor_scalar_mul(
                    ctile[:, WW + 2 * XB :], ctile[:, WW + XB : WW + 2 * XB], -1.0
                )
                wv = ctile[:, :WW].rearrange("p (c y o) -> p c y o", c=2, y=yc)
                xv = ctile[:, WW:].rearrange("p (c y b) -> p c y b", c=3, y=yc)
                # otile col = y*64 + c*32 + b: exactly the psum layout, so the
                # copyback is a flat contiguous copy; host untangles for free.
                # One otile spans two x-slices so out-DMAs are 2x larger.
                if x % 2 == 0:
                    otile = opool.tile([cout, 2 * OW], dt, name="otile")
                obase = (x % 2) * OW
                for y0, gs in groups:
                    ptile = ppool.tile([cout, 512], f32, name="ptile")
                    for m in range(gs):
                        y = y0 + m
                        ps = ptile[:, m * 2 * b : (m + 1) * 2 * b]
                        nc.tensor.matmul(
                            ps, wv[:, 0, y, :], xv[:, 0:2, y, :],
                            start=True, stop=False,
                        )
                        nc.tensor.matmul(
                            ps, wv[:, 1, y, :], xv[:, 2::-2, y, :],
                            start=False, stop=True,
                        )
                    nc.vector.tensor_copy(
                        out=otile[:, obase + y0 * 2 * b : obase + (y0 + gs) * 2 * b],
                        in_=ptile[:, : gs * 2 * b],
                    )
                # out-DMAs ride the GPSIMD SWDGE queue: SP keeps input DMAs,
                # ACT keeps the negates (else a waiting out-DMA head-of-line
                # blocks the next negate and stalls PE); the last pair is
                # split so slice 6's output ships while slice 7 computes
                if x == xpc - 2 and xpc >= 2:
                    nc.gpsimd.dma_start(out=out[:, x // 2, :OW], in_=otile[:, :OW])
                elif x == xpc - 1 and xpc >= 2:
                    nc.gpsimd.dma_start(out=out[:, x // 2, OW:], in_=otile[:, OW:])
                elif x % 2 == 1:
                    d = nc.gpsimd.dma_start(out=out[:, x // 2, :], in_=otile)
                    if x >= 3:
                        # HBM is read-saturated until the last input lands, so
                        # sending these earlier only delays the final input;
                        # defer them to fill the pipe while slice 7 computes
                        deferred.append(d)
            for d in deferred:
                add_dep_helper(d.ins, in_dmas[-1].ins, True, "pack outs after last in")

    _split_excess_waits(nc, mybir)
    return nc


def prep_inputs(input, weight):
    """Host-side re-layout + fp16 cast. Returns combined [64, 128, 16640+4160]."""
    # weight [i,o,x,y,c] -> [x,i,c,y,o]
    w16 = weight.transpose(2, 0, 4, 3, 1).astype(np.float16)
    w16 = w16.reshape(M1, CIN, 2 * M2 * COUT)
    xr = input[..., 0]
    xi = input[..., 1]
    st = np.stack([xr, xi], axis=0)  # [c,b,i,x,y]
    x16 = st.transpose(3, 2, 0, 4, 1).astype(np.float16)  # [x,i,c,y,b]
    x16 = x16.reshape(M1, CIN, 2 * B * M2)
    return np.concatenate([w16, x16], axis=2)


def gather_output(per_core):
    """per_core: list of 8 arrays [cout, xpc//2, 2*yc*2*b] fp16 -> [B, COUT, M1, M2, 2] f32."""
    out = np.empty((B, COUT, M1, M2, 2), np.float32)
    for k, arr in enumerate(per_core):
        a = arr.reshape(COUT, XPC, M2, 2, B)  # [o, x, y, c, b]
        out[:, :, k * XPC : (k + 1) * XPC] = a.transpose(4, 0, 1, 2, 3)
    return out


_NC = None
TRACE = False  # test harness can set True to collect a HW profile
LAST_RESULTS = None


def kernel(input, weight):
    global _NC, LAST_RESULTS
    from concourse.bass_utils import run_bass_kernel_spmd

    if _NC is None:
        _NC = build_nc()
    c16 = prep_inputs(np.asarray(input), np.asarray(weight))
    in_maps = [
        {"cin": np.ascontiguousarray(c16[k * XPC : (k + 1) * XPC])}
        for k in range(NCORES)
    ]
    res = run_bass_kernel_spmd(_NC, in_maps, core_ids=list(range(NCORES)), trace=TRACE)
    LAST_RESULTS = res
    return gather_output([r["out"] for r in res.results])


# revision 23
# speedup vs baseline: 1.0540x; 1.0005x over previous
"""Complex per-mode matmul: out[b,o,x,y] = sum_i in[b,i,x,y] * w[i,o,x,y] (complex).

Shapes (hardcoded): input [32,128,64,65,2] f32, weight [128,128,64,65,2] f32,
output [32,128,64,65,2] f32, where the trailing 2 is (real, imag).

Strategy:
  - Shard the 64 x-modes across 8 cores (8 per core). Contraction is over
    in_channels for each (x,y) independently, so this needs zero replication
    and no collectives: per-core I/O is 1/8 of everything.
  - Per mode (x,y): psum[o, c*32+b] accumulates two matmuls
        MM1: lhsT=Wr[i,o] (128 cols), rhs cols (Xr[i,b] | Xi[i,b]) blocks -> (Wr.Xr | Wr.Xi)
        MM2: lhsT=Wi[i,o],           rhs cols (-Xi[i,b] | Xr[i,b]) blocks -> (-Wi.Xi | Wi.Xr)
    giving out_r = Wr.Xr - Wi.Xi in the low 32 cols, out_i = Wr.Xi + Wi.Xr in
    the high 32. The -Xi block is produced on-device by one DVE scalar-mul per
    x-slice (cheaper than shipping a third X copy from HBM).
  - Inputs fp16 (PSUM accumulates fp32); output stored fp16 and upcast on host.
  - Host pre-transposes operands so every DMA moves large contiguous
    per-partition lines:
      cin layout [x][i (part)][w: c(2),y(65),o(128) | xr: y(65),b(32) | xi: y(65),b(32)]
      out layout [o (part)][x][b(32), y(65), c(2)]   fp16
  - This walrus build fits only ONE sync wait per hardware instruction; a
    post-pass splits any extra waits into standalone EventSemaphore
    instructions on the same engine queue (the wait-carrier bacc uses).
"""

import numpy as np

B, CIN, COUT, M1, M2 = 32, 128, 128, 64, 65
NCORES = 8
XPC = M1 // NCORES  # x-slices per core
MPG = 8  # modes per PSUM bank (8 * 64 cols = 512 = one bank)


def _split_excess_waits(nc, mybir):
    """Walrus codegen fits one sync wait per instruction; move extras onto
    EventSemaphore instructions inserted just before, on the same engine."""
    n = 0
    for fn in nc.m.functions:
        for blk in fn.blocks:
            out = []
            for inst in blk.instructions:
                si = inst.sync_info
                if si is not None and si.on_wait and len(si.on_wait) > 1:
                    waits = list(si.on_wait)
                    for w in waits[:-1]:
                        ev = mybir.InstEventSemaphore(
                            name=f"evsplit_{n}",
                            engine=inst.engine,
                            ins=[],
                            outs=[],
                            sync_info=mybir.SyncInfo(on_wait=[w], on_update=[]),
                            bass_nofuse=True,
                        )
                        n += 1
                        nc.register_instruction(ev)
                        out.append(ev)
                    si.on_wait = [waits[-1]]
                out.append(inst)
            blk.instructions = out


def build_nc(xpc=XPC, b=B, yc=M2, cout=COUT):
    import concourse.bass as bass
    import concourse.mybir as mybir
    from concourse.tile import TileContext
    from concourse.tile_rust import add_dep_helper

    dt = mybir.dt.float16
    f32 = mybir.dt.float32
    WW = 2 * yc * cout  # weight cols per slice
    XB = b * yc  # one x block (b, y)
    nc = bass.Bass()
    cin = nc.dram_tensor("cin", [xpc, CIN, WW + 2 * XB], dt, kind="ExternalInput")
    out = nc.dram_tensor("out", [cout, max(1, xpc // 2), 2 * b * yc * 2], dt, kind="ExternalOutput")

    groups = [(g0, min(MPG, yc - g0)) for g0 in range(0, yc, MPG)]

    with TileContext(nc) as tc:
        with (
            tc.tile_pool(name="cpool", bufs=2) as cpool,
            tc.tile_pool(name="opool", bufs=3) as opool,
            tc.tile_pool(name="ppool", bufs=4, space="PSUM") as ppool,
        ):
            OW = b * yc * 2  # out cols per slice
            otile = None
            in_dmas = []
            deferred = []
            for x in range(xpc):
                # ctile: [ w (WW) | xr (XB) | xi (XB) | -xi (XB, ACT-written) ]
                ctile = cpool.tile([CIN, WW + 3 * XB], dt, name="ctile")
                in_dmas.append(nc.sync.dma_start(out=ctile[:, : WW + 2 * XB], in_=cin[x]))
                # -xi block on DVE: using ScalarE here pulls in its activation
                # tables as ~8us of static DMA before the kernel starts
                nc.vector.tensor_scalar_mul(
                    ctile[:, WW + 2 * XB :], ctile[:, WW + XB : WW + 2 * XB], -1.0
                )
                wv = ctile[:, :WW].rearrange("p (c y o) -> p c y o", c=2, y=yc)
                xv = ctile[:, WW:].rearrange("p (c y b) -> p c y b", c=3, y=yc)
                # otile col = y*64 + c*32 + b: exactly the psum layout, so the
                # copyback is a flat contiguous copy; host untangles for free.
                # One otile spans two x-slices so out-DMAs are 2x larger.
                if x % 2 == 0:
                    otile = opool.tile([cout, 2 * OW], dt, name="otile")
                obase = (x % 2) * OW
                tail = x >= xpc - 2 and xpc >= 2
                half = len(groups) // 2  # groups below this ship early on tail slices
                hcol = obase + groups[half][0] * 2 * b if tail else None
                for gidx, (y0, gs) in enumerate(groups):
                    ptile = ppool.tile([cout, 512], f32, name="ptile")
                    for m in range(gs):
                        y = y0 + m
                        ps = ptile[:, m * 2 * b : (m + 1) * 2 * b]
                        nc.tensor.matmul(
                            ps, wv[:, 0, y, :], xv[:, 0:2, y, :],
                            start=True, stop=False,
                        )
                        nc.tensor.matmul(
                            ps, wv[:, 1, y, :], xv[:, 2::-2, y, :],
                            start=False, stop=True,
                        )
                    nc.vector.tensor_copy(
                        out=otile[:, obase + y0 * 2 * b : obase + (y0 + gs) * 2 * b],
                        in_=ptile[:, : gs * 2 * b],
                    )
                    if tail and gidx == half - 1:
                        # first half of a tail slice ships while its second
                        # half computes, shortening the end-of-kernel chain
                        nc.gpsimd.dma_start(
                            out=out[:, x // 2, obase:hcol],
                            in_=otile[:, obase:hcol],
                        )
                # out-DMAs ride the GPSIMD SWDGE queue: SP keeps input DMAs,
                # ACT keeps the negates (else a waiting out-DMA head-of-line
                # blocks the next negate and stalls PE); the last pair is
                # split so slice 6's output ships while slice 7 computes
                if tail:
                    nc.gpsimd.dma_start(
                        out=out[:, x // 2, hcol : obase + OW], in_=otile[:, hcol : obase + OW]
                    )
                elif x % 2 == 1:
                    d = nc.gpsimd.dma_start(out=out[:, x // 2, :], in_=otile)
                    if x >= 3:
                        # HBM is read-saturated until the last input lands, so
                        # sending these earlier only delays the final input;
                        # defer them to fill the pipe while slice 7 computes
                        deferred.append(d)
            for d in deferred:
                add_dep_helper(d.ins, in_dmas[-1].ins, True, "pack outs after last in")

    _split_excess_waits(nc, mybir)
    return nc


def prep_inputs(input, weight):
    """Host-side re-layout + fp16 cast. Returns combined [64, 128, 16640+4160]."""
    # weight [i,o,x,y,c] -> [x,i,c,y,o]
    w16 = weight.transpose(2, 0, 4, 3, 1).astype(np.float16)
    w16 = w16.reshape(M1, CIN, 2 * M2 * COUT)
    xr = input[..., 0]
    xi = input[..., 1]
    st = np.stack([xr, xi], axis=0)  # [c,b,i,x,y]
    x16 = st.transpose(3, 2, 0, 4, 1).astype(np.float16)  # [x,i,c,y,b]
    x16 = x16.reshape(M1, CIN, 2 * B * M2)
    return np.concatenate([w16, x16], axis=2)


def gather_output(per_core):
    """per_core: list of 8 arrays [cout, xpc//2, 2*yc*2*b] fp16 -> [B, COUT, M1, M2, 2] f32."""
    out = np.empty((B, COUT, M1, M2, 2), np.float32)
    for k, arr in enumerate(per_core):
        a = arr.reshape(COUT, XPC, M2, 2, B)  # [o, x, y, c, b]
        out[:, :, k * XPC : (k + 1) * XPC] = a.transpose(4, 0, 1, 2, 3)
    return out


_NC = None
TRACE = False  # test harness can set True to collect a HW profile
LAST_RESULTS = None


def kernel(input, weight):
    global _NC, LAST_RESULTS
    from concourse.bass_utils import run_bass_kernel_spmd

    if _NC is None:
        _NC = build_nc()
    c16 = prep_inputs(np.asarray(input), np.asarray(weight))
    in_maps = [
        {"cin": np.ascontiguousarray(c16[k * XPC : (k + 1) * XPC])}
        for k in range(NCORES)
    ]
    res = run_bass_kernel_spmd(_NC, in_maps, core_ids=list(range(NCORES)), trace=TRACE)
    LAST_RESULTS = res
    return gather_output([r["out"] for r in res.results])
